# revision 94
# baseline (speedup 1.0000x reference)
"""Multi-head attention Trainium2 kernel (B=4, T=1024, C=1024, H=16, D=64).

Sharding over 8 NeuronCores: core c handles batch b = c//2 and head group
g = c%2 (heads [8g, 8g+8)).  Each core computes a partial out-projection
(its 8 heads' contribution, [T, C]); the host sums the two partials per
batch and adds b_out (plus the folded V-bias term bv @ W_out).  No
on-device collectives.

All matmul operands are bf16 (PSUM accumulation stays fp32); weights and
x are pre-packed on the host into SBUF layout so every DMA moves >=2KB
contiguous per partition (few large transfers — HWDGE descriptor-gen and
the DMA engines are serialized resources).

Math (per core):
  XT[p, ki, t]  host-packed x[b].T
  QT/KT[f, t] = Wqk[:, f].T @ XT     (pair-stacked [128, T], Q pre-scaled
                                      1/8 on host, bias via DVE add)
  V[t, f]     = XT-chunk.T @ Wv      (natural layout, ones col appended)
  S^T[k, q]   = KT-slice.T @ QT-slice  (per head, causal blocks only)
  P           = exp(S^T)  (ACT, bf16), tri mask on diag blocks (DVE)
  vq[q, h, d|s] = P_h-block.T @ [V_h | 1]   (P stationary, one matmul
                  per head per causal block: free dim is only 65 wide, so
                  attnV costs 65 cycles/block instead of 128; the softmax
                  denominator s lands PER PARTITION q)
  vals        = vq[:, :, 0:64] * recip(s)[q, h]  (free-dim broadcast —
                  no cross-partition broadcast needed at all)
  vals^T      = PE transpose per (pair, q-tile)  (identity operand)
  out[q, c]   = vals^T.T @ Wout-slice

Schedule (the in-order PE queue is the bottleneck, ~85us of matmul at
2.4GHz — every matmul must be ready when the queue reaches it):
  - dead "warmup" matmuls hold the PE p-state ramp while x streams in,
    and fill the DMA-paced gaps of the first two QKV slots;
  - all 48 score tiles are woven through the QKV/V phase at ~1 tile per
    1.2us so the 2-deep score-PSUM ring never stalls on ACT's exp
    throughput (37us total, the busiest non-PE engine);
  - phase 2 runs one attnV block per 128-query tile (all 8 heads into
    two half-bank PSUM tiles, 4-deep ring), normalizes with a [128,8]
    reciprocal + one broadcast-multiply per half, transposes, and weaves
    the out-projection chunks behind the transposes they consume.
"""

import numpy as np
import ml_dtypes

import concourse.mybir as mybir
import concourse.tile as tile
from concourse import bacc
from concourse.bass_utils import run_bass_kernel_spmd

B, T, C, H, D = 4, 1024, 1024, 16, 64
P = 128            # partitions
HPC = 8            # heads per core
PAIRS = 4          # head pairs per core
NK = C // P        # 8 contraction tiles
KT_TILES = T // P  # 8 k-tiles over sequence
QC = 512           # q-chunk (PSUM bank free size, fp32)
NQC = T // QC      # 2 q-chunks
F32 = mybir.dt.float32
BF16 = mybir.dt.bfloat16
AF = mybir.ActivationFunctionType
ALU = mybir.AluOpType

_CACHE = {}


def _build_nc():
    nc = bacc.Bacc(None, target_bir_lowering=False)

    xT = nc.dram_tensor("xT", [P, NK, T], BF16, kind="ExternalInput")
    wqk01 = nc.dram_tensor("wqk01", [P, 2, NK, P], BF16, kind="ExternalInput")
    wqk23 = nc.dram_tensor("wqk23", [P, 2, NK, P], BF16, kind="ExternalInput")
    wqk47 = nc.dram_tensor("wqk47", [P, 4, NK, P], BF16, kind="ExternalInput")
    wv = nc.dram_tensor("wv", [P, NK, HPC * D], BF16, kind="ExternalInput")
    wout = nc.dram_tensor("wout", [P, PAIRS, C], BF16, kind="ExternalInput")
    bqk = nc.dram_tensor("bqk", [P, 8], F32, kind="ExternalInput")
    tri2 = nc.dram_tensor("tri2", [P, 2, P], BF16, kind="ExternalInput")
    ident = nc.dram_tensor("ident", [P, P], BF16, kind="ExternalInput")
    out = nc.dram_tensor("out", [T, C], BF16, kind="ExternalOutput")

    with tile.TileContext(nc) as tc:
        with (
            tc.tile_pool(name="consts", bufs=1) as consts,
            tc.tile_pool(name="wqk_p", bufs=1) as wqk_pool,
            tc.tile_pool(name="qkt", bufs=8) as qkt_pool,
            tc.tile_pool(name="vsb", bufs=8) as v_pool,
            tc.tile_pool(name="probs", bufs=48) as p_pool,
            tc.tile_pool(name="vals", bufs=32) as vals_pool,
            tc.tile_pool(name="smal", bufs=2) as s2_pool,
        ):
            # ---- warmup scratch (tiny memset so it has a writer) ----
            warm_sb = consts.tile([P, P], BF16)
            nc.vector.memset(warm_sb, 0.0)

            # ---- input DMAs (SP queue, arrival-critical order; few, large
            # transfers — HWDGE descriptor-gen is serialized at 625ns each)
            xt_ctx = tc.tile_pool(name="xt", bufs=1)
            xt_pool = xt_ctx.__enter__()
            w_all = wqk_pool.tile([P, 8, NK, P], BF16)
            nc.sync.dma_start(w_all[:, 0:2], wqk01[:, :, :, :])
            xt_sb = xt_pool.tile([P, NK, T], BF16)
            nc.sync.dma_start(xt_sb[:, 0, :], xT[:, 0, :])
            bqk_sb = consts.tile([P, 8], F32)
            nc.sync.dma_start(bqk_sb, bqk[:, :])
            tri2_sb = consts.tile([P, 2, P], BF16)
            nc.sync.dma_start(tri2_sb, tri2[:, :, :])
            for ki in range(1, NK):
                nc.sync.dma_start(xt_sb[:, ki, :], xT[:, ki, :])
            ident_sb = consts.tile([P, P], BF16)
            nc.sync.dma_start(ident_sb, ident[:, :])
            nc.sync.dma_start(w_all[:, 2:4], wqk23[:, :, :, :])
            nc.sync.dma_start(w_all[:, 4:8], wqk47[:, :, :, :])
            wv_sb = consts.tile([P, NK, HPC * D], BF16)
            nc.sync.dma_start(wv_sb, wv[:, :, :])
            wout_sb = consts.tile([P, PAIRS, C], BF16)
            nc.sync.dma_start(wout_sb, wout[:, :, :])

            # persistent v tiles; ones column memset early (no deps)
            v_sb = []
            for ti in range(KT_TILES):
                vt = v_pool.tile([P, HPC, D + 1], BF16, tag="v_sb", name=f"v{ti}")
                v_sb.append(vt)
                nc.vector.memset(vt[:, :, D : D + 1], 1.0)

            qkt_sb = {}

            # ---- phase 1 pools: QKV psum + scores psum = 4 + 4 banks ----
            p1_ctx = tc.tile_pool(name="qkv_ps", bufs=2, space="PSUM")
            p1 = p1_ctx.__enter__()
            p2_ctx = tc.tile_pool(name="s_ps", bufs=2, space="PSUM")
            p2 = p2_ctx.__enter__()

            def emit_warm_mms(ps, n):
                """Dead matmuls into the (not yet used) qc1 half of a slot's
                PSUM tile; the real qc1 accumulation's start flag clears it.
                Holds the PE p-state ramp while input DMAs land."""
                for _ in range(n):
                    nc.tensor.matmul(
                        ps[:, 1, 0:P], warm_sb[:, 0:P], warm_sb[:, 0:P],
                        start=True, stop=True, skip_group_check=True,
                    )

            _slot_ps = {}

            def emit_slot_half(slot, qc, warm=0):
                """Half of QT/KT slot: one q-chunk accumulation.  warm:
                interleave dead matmuls after each ki so the xt-DMA-paced
                start never leaves a PE gap (keeps the p-state ramp alive)."""
                if qc == 0 and slot not in _slot_ps:
                    _slot_ps[slot] = p1.tile(
                        [P, NQC, QC], F32, tag="qkv", name=f"qkvps{slot}"
                    )
                ps = _slot_ps[slot]
                for ki in range(NK):
                    nc.tensor.matmul(
                        ps[:, qc, :],
                        w_all[:, slot, ki, :],
                        xt_sb[:, ki, qc * QC : (qc + 1) * QC],
                        start=(ki == 0),
                        stop=(ki == NK - 1),
                    )
                    if warm and qc == 0:
                        emit_warm_mms(ps, warm)
                if qc == NQC - 1:
                    dst = qkt_pool.tile([P, T], BF16, tag="qkt", name=f"qkt{slot}")
                    qkt_sb[slot] = dst
                    nc.vector.tensor_scalar_add(
                        dst,
                        ps.rearrange("p a b -> p (a b)"),
                        bqk_sb[:, slot : slot + 1],
                    )

            def emit_v(ti):
                ps = p1.tile([P, QC], F32, tag="qkv", name=f"vps{ti}")
                for ki in range(NK):
                    nc.tensor.matmul(
                        ps,
                        xt_sb[:, ki, ti * P : (ti + 1) * P],
                        wv_sb[:, ki, :],
                        start=(ki == 0),
                        stop=(ki == NK - 1),
                    )
                nc.vector.tensor_copy(
                    v_sb[ti][:, :, 0:D], ps.rearrange("p (h d) -> p h d", h=HPC)
                )

            p_tiles = {}  # (pair, qc, kj) -> P tile [128, 2, QC] bf16

            def emit_sc(pair, qc, kjs):
                """Score tiles + exp (+ tri mask on diagonal blocks)."""
                qt = qkt_sb[2 * pair]
                kt = qkt_sb[2 * pair + 1]
                for kj in kjs:
                    j0 = kj - 4 * qc
                    q_lo = max(j0, 0) * P
                    sps = p2.tile([P, 2, QC], F32, tag="s", name="sps")
                    for hl in range(2):
                        d0 = D * hl
                        nc.tensor.matmul(
                            sps[:, hl, q_lo:QC],
                            kt[d0 : d0 + D, kj * P : (kj + 1) * P],
                            qt[d0 : d0 + D, qc * QC + q_lo : (qc + 1) * QC],
                            start=True,
                            stop=True,
                        )
                    pt = p_pool.tile([P, 2, QC], BF16, tag="probs")
                    p_tiles[(pair, qc, kj)] = pt
                    nc.scalar.activation(
                        pt[:, :, q_lo:QC], sps[:, :, q_lo:QC], AF.Exp
                    )
                    if j0 >= 0:
                        nc.vector.tensor_tensor(
                            pt[:, :, q_lo : q_lo + P],
                            pt[:, :, q_lo : q_lo + P],
                            tri2_sb[:, :, :],
                            ALU.mult,
                        )

            vals_T = {}

            def emit_avq(jq, pq, ptp):
                """attnV for one q-tile of 128, all 8 heads, P-stationary:
                out[q, h, d(+s)] = P_h[k, q].T @ [V_h | 1].  The softmax
                denominator lands per PARTITION (q), so normalization is a
                native free-dim-broadcast multiply — no partition broadcast,
                no cross-engine chain.  Then per-pair PE transposes put vals
                back into [hd, q] for the out-projection lhsT."""
                qc, tsub = jq // 4, jq % 4
                q0 = tsub * P
                n_kj = jq + 1
                halves = []
                for half in range(2):
                    vq = pq.tile(
                        [P, 4, D + 1], F32, tag="vq", name=f"vq{jq}_{half}"
                    )
                    halves.append(vq)
                    for hh in range(4):
                        h_abs = 4 * half + hh
                        pair, hl = h_abs // 2, h_abs % 2
                        for kj in range(n_kj):
                            nc.tensor.matmul(
                                vq[:, hh, :],
                                p_tiles[(pair, qc, kj)][:, hl, q0 : q0 + P],
                                v_sb[kj][:, h_abs, :],
                                start=(kj == 0),
                                stop=(kj == n_kj - 1),
                                skip_group_check=True,
                            )
                # normalize: s sits at free-index 64 per (q, h) — recip of
                # a [128, 8] gather, then scale with free-dim broadcast
                s8 = s2_pool.tile([P, 8], F32, tag="s8")
                nc.vector.tensor_copy(s8[:, 0:4], halves[0][:, :, D])
                nc.vector.tensor_copy(s8[:, 4:8], halves[1][:, :, D])
                r8 = s2_pool.tile([P, 8], F32, tag="r8")
                nc.vector.reciprocal_approx_fast(r8, s8)
                vqn = vun_pool.tile([P, 8, D], BF16, tag="vqn", name=f"vqn{jq}")
                for half in range(2):
                    nc.vector.tensor_tensor(
                        vqn[:, 4 * half : 4 * half + 4, :],
                        halves[half][:, :, 0:D],
                        r8[:, 4 * half : 4 * half + 4, None].to_broadcast(
                            [P, 4, D]
                        ),
                        ALU.mult,
                    )
                # transpose each pair's [128 q, 128 hd] block back to [hd, q]
                for pair in range(PAIRS):
                    tp = ptp.tile([P, P], BF16, tag="tp", name=f"tp{jq}_{pair}")
                    nc.tensor.transpose(
                        tp,
                        vqn[:, 2 * pair : 2 * pair + 2, :].rearrange(
                            "p a b -> p (a b)"
                        ),
                        ident_sb,
                    )
                    vt = vals_pool.tile(
                        [P, P], BF16, tag="vals", name=f"vT{jq}_{pair}"
                    )
                    vals_T[(pair, jq)] = vt
                    nc.vector.tensor_copy(vt, tp)

            _oq_ps = {}

            def emit_oq(qc, tsub, cc, p4, pairs, copy_eng):
                """Out-projection chunk [128 q, 512 c]; `pairs` may split the
                accumulation across calls (last call finishes + stores)."""
                q0 = tsub * P
                key = (qc, tsub, cc)
                if key not in _oq_ps:
                    _oq_ps[key] = p4.tile(
                        [P, QC], F32, tag="ops", name=f"ops{qc}_{tsub}_{cc}"
                    )
                ops = _oq_ps[key]
                for pair in pairs:
                    nc.tensor.matmul(
                        ops,
                        vals_T[(pair, qc * 4 + tsub)],
                        wout_sb[:, pair, cc * QC : (cc + 1) * QC],
                        start=(pair == 0),
                        stop=(pair == PAIRS - 1),
                        skip_group_check=True,
                    )
                if pairs[-1] != PAIRS - 1:
                    return
                o_sb = out_pool.tile([P, QC], BF16, tag="o_sb")
                if copy_eng == "act":
                    nc.scalar.activation(o_sb, ops, AF.Copy)
                else:
                    nc.vector.tensor_copy(o_sb, ops)
                nc.sync.dma_start(
                    out[qc * QC + q0 : qc * QC + q0 + P, cc * QC : (cc + 1) * QC],
                    o_sb,
                )

            # ---- phase 1: QKV + V matmuls with all score tiles woven in.
            # Weave rule: ~1 score tile (0.85us PE, 1us ACT exp) per ~1.2us
            # of independent filler matmuls, so the 2-deep score-PSUM ring
            # never stalls the in-order PE queue.  qc0 pairs early (attnV
            # consumes them first in phase 2), pair 3 qc1 last.
            _slot_ps[0] = p1.tile([P, NQC, QC], F32, tag="qkv", name="qkvps0")
            emit_warm_mms(_slot_ps[0], 34)
            emit_slot_half(0, 0, warm=2)
            emit_slot_half(1, 0, warm=2)
            emit_slot_half(0, 1)
            emit_slot_half(1, 1)
            emit_slot_half(2, 0)
            emit_sc(0, 0, [0, 1, 2])
            emit_slot_half(2, 1)
            emit_sc(0, 0, [3])
            emit_sc(0, 1, [0, 1])
            emit_slot_half(3, 0)
            emit_sc(0, 1, [2, 3, 4])
            emit_slot_half(3, 1)
            emit_sc(0, 1, [5, 6, 7])
            emit_slot_half(4, 0)
            emit_sc(1, 0, [0, 1, 2])
            emit_slot_half(4, 1)
            emit_sc(1, 0, [3])
            emit_sc(1, 1, [0, 1])
            emit_slot_half(5, 0)
            emit_sc(1, 1, [2, 3, 4])
            emit_slot_half(5, 1)
            emit_sc(1, 1, [5, 6, 7])
            emit_v(0)
            emit_sc(2, 0, [0])
            emit_v(1)
            emit_sc(2, 0, [1, 2])
            emit_v(2)
            emit_sc(2, 0, [3])
            emit_v(3)
            emit_sc(2, 1, [0, 1])
            emit_slot_half(6, 0)
            emit_sc(2, 1, [2, 3, 4])
            emit_slot_half(6, 1)
            emit_sc(2, 1, [5, 6, 7])
            emit_slot_half(7, 0)
            emit_slot_half(7, 1)
            emit_sc(3, 0, [0, 1, 2])
            emit_v(4)
            emit_sc(3, 0, [3])
            emit_sc(3, 1, [0, 1])
            emit_v(5)
            emit_sc(3, 1, [2, 3])
            emit_sc(3, 1, [4, 5])
            emit_v(6)
            emit_sc(3, 1, [6, 7])
            emit_v(7)

            p2_ctx.__exit__(None, None, None)
            p1_ctx.__exit__(None, None, None)
            xt_ctx.__exit__(None, None, None)
            vun_ctx = tc.tile_pool(name="vun", bufs=3)
            vun_pool = vun_ctx.__enter__()
            outs_ctx = tc.tile_pool(name="outs", bufs=3)
            out_pool = outs_ctx.__enter__()

            # ---- phase 2: P-stationary attnV per q-tile (4+2+2 PSUM
            # banks), out-projection woven behind the transposes ----
            p3_ctx = tc.tile_pool(name="vq_ps", bufs=4, space="PSUM")
            pq = p3_ctx.__enter__()
            ptp_ctx = tc.tile_pool(name="tp_ps", bufs=2, space="PSUM")
            ptp = ptp_ctx.__enter__()
            p4_ctx = tc.tile_pool(name="o_ps", bufs=2, space="PSUM")
            p4 = p4_ctx.__enter__()

            for jq in range(4):
                emit_avq(jq, pq, ptp)
            emit_avq(4, pq, ptp)
            for cc in range(2):
                emit_oq(0, 0, cc, p4, [0, 1, 2, 3], "act")
            emit_avq(5, pq, ptp)
            for cc in range(2):
                emit_oq(0, 1, cc, p4, [0, 1, 2, 3], "act")
            emit_avq(6, pq, ptp)
            for cc in range(2):
                emit_oq(0, 2, cc, p4, [0, 1, 2, 3], "act")
            emit_avq(7, pq, ptp)
            for cc in range(2):
                emit_oq(0, 3, cc, p4, [0, 1, 2, 3], "act")
            for tsub in range(4):
                for cc in range(2):
                    emit_oq(1, tsub, cc, p4, [0, 1, 2, 3], "act")

            p4_ctx.__exit__(None, None, None)
            ptp_ctx.__exit__(None, None, None)
            p3_ctx.__exit__(None, None, None)
            outs_ctx.__exit__(None, None, None)
            vun_ctx.__exit__(None, None, None)

    nc.compile()
    return nc


def _host_shards(x, mask, W_in, b_in, W_out, b_out):
    """Build the 8 per-core input maps (bf16, SBUF-packed layouts)."""
    del mask  # causal structure is hardcoded (tri2 built locally)
    x = np.asarray(x, dtype=np.float32)
    W_in = np.asarray(W_in, dtype=np.float32)
    b_in = np.asarray(b_in, dtype=np.float32)
    W_out = np.asarray(W_out, dtype=np.float32)
    bf = ml_dtypes.bfloat16

    tri = np.triu(np.ones((P, P), dtype=np.float32))  # tri[k, q] = 1 if k <= q
    ident = np.eye(P, dtype=np.float32).astype(bf)
    tri2 = np.ascontiguousarray(
        np.broadcast_to(tri[:, None, :], (P, 2, P))
    ).astype(bf)
    xTs = [
        np.ascontiguousarray(
            x[b].T.reshape(NK, P, T).transpose(1, 0, 2)
        ).astype(bf)
        for b in range(B)
    ]

    per_group = {}
    for g in range(2):
        wqk = np.empty((8, P, NK, P), dtype=np.float32)
        bqk = np.empty((P, 8), dtype=np.float32)
        for p in range(PAIRS):
            qcols = slice((8 * g + 2 * p) * D, (8 * g + 2 * p + 2) * D)
            kcols = slice(C + (8 * g + 2 * p) * D, C + (8 * g + 2 * p + 2) * D)
            wqk[2 * p] = (
                W_in[:, qcols].reshape(NK, P, P).transpose(1, 0, 2) * 0.125
            )
            wqk[2 * p + 1] = W_in[:, kcols].reshape(NK, P, P).transpose(1, 0, 2)
            bqk[:, 2 * p] = b_in[qcols] * 0.125
            bqk[:, 2 * p + 1] = b_in[kcols]
        vcols = slice(2 * C + g * 512, 2 * C + (g + 1) * 512)
        wv = np.ascontiguousarray(
            W_in[:, vcols].reshape(NK, P, 512).transpose(1, 0, 2)
        ).astype(bf)
        wout = np.ascontiguousarray(
            W_out[g * 512 : (g + 1) * 512, :]
            .reshape(PAIRS, P, C)
            .transpose(1, 0, 2)
        ).astype(bf)
        wqk_p = wqk.transpose(1, 0, 2, 3)  # [p, slot, ki, f]
        per_group[g] = dict(
            wqk01=np.ascontiguousarray(wqk_p[:, 0:2]).astype(bf),
            wqk23=np.ascontiguousarray(wqk_p[:, 2:4]).astype(bf),
            wqk47=np.ascontiguousarray(wqk_p[:, 4:8]).astype(bf),
            bqk=bqk, wv=wv, wout=wout, tri2=tri2, ident=ident,
        )

    in_maps = []
    for c in range(8):
        b, g = c // 2, c % 2
        m = dict(per_group[g])
        m["xT"] = xTs[b]
        in_maps.append(m)
    return in_maps


def run(inputs, trace=False):
    if "nc" not in _CACHE:
        _CACHE["nc"] = _build_nc()
    nc = _CACHE["nc"]
    in_maps = _host_shards(**inputs)
    res = run_bass_kernel_spmd(
        nc, in_maps, core_ids=list(range(8)), trace=trace,
        trace_cores=list(range(8)) if trace else None,
    )
    b_in = np.asarray(inputs["b_in"], dtype=np.float32)
    W_out = np.asarray(inputs["W_out"], dtype=np.float32)
    b_out = np.asarray(inputs["b_out"], dtype=np.float32)
    # V-bias folded out of the device kernel: vals_true = vals_dev + b_v,
    # so out_true = out_dev + b_v @ W_out (+ b_out), added once per batch.
    bias = b_out + b_in[2 * C :] @ W_out
    out = np.empty((B, T, C), dtype=np.float32)
    for b in range(B):
        out[b] = (
            np.asarray(res.results[2 * b]["out"], dtype=np.float32)
            + np.asarray(res.results[2 * b + 1]["out"], dtype=np.float32)
            + bias
        )
    return out, res


def kernel(**inputs) -> np.ndarray:
    out, _ = run(inputs, trace=False)
    return out
